# revision 26
# baseline (speedup 1.0000x reference)
"""Trainium2 Bass kernel for causal multi-head attention (fp8 DoubleRow).

Problem: x[4, 2048, 1024] -> Attention(heads=16, causal) -> out[4, 2048, 1024]

Sharding over 8 NeuronCores: core c handles batch bi = c // 2 and head-half
hj = c % 2 (8 of the 16 heads).  Each core computes its 8 heads' attention
and a partial output projection (row-parallel Wo); the host sums the two
partials per batch element and adds bv@Wo + bo.

Numerical split (tolerance is 2e-2 of max|y|; short causal rows are the
error-sensitive ones, long rows average fp8 noise away):
  - Tokens 0-511 everywhere they can feed a short row stay fp16: Q/K/V
    projections for chunk 0 / j-tiles 0-3, chunk-0 S, chunk-0 PV.
  - Tokens 512+ projections run fp8(e4m3) DoubleRow (2 contraction tiles
    per instruction -> 2x PE throughput).
  - Chunks 1-3 PV runs fp8 DoubleRow pairing two j-tiles per matmul;
    pt = exp(s/8 - 3) written by ScalarE directly in e4m3 (bias -3 keeps
    e^s <= ~90 < 240), V' = [V | 1 | 0-pad] padded to 80 cols (DR weight
    free-dim must be 16B-aligned; DR output must start at partition 0).
    The uncomputed below-diagonal strip of each DR pair's second tile is
    gpsimd-memset to 0.
  - One in three full j-tile pairs routes its exp to DVE as a Schraudolph
    bit-trick (one tensor_scalar producing e5m2 bits; negative bits only
    at |s| > 7 sigma, where the fp32->uint8 conversion saturates to 0),
    offloading ScalarE, the attention-phase pace-setter.
  - S stays fp16 (K=64 row-packed pairs already run at ~1 col/cycle for
    both heads; fp8 adds nothing), out-projection stays fp16 (fp8 there
    fails the error budget).

The emission schedule (fill architecture) is inherited from the fp16
kernel: projection work streams through attention fill slots so the PE
never starves while ScalarE catches up on exp.  Engine budget per core:
PE ~205us busy, ScalarE ~120us (exp), DVE ~125us (evictions, staging,
normalize, Schraudolph), GpSimd ~80us (affine masks, broadcasts).
"""

import os
import sys

for _p in ("/opt/trn_rl_repo",):
    if _p not in sys.path and os.path.isdir(_p):
        sys.path.insert(0, _p)

import numpy as np
import ml_dtypes

import concourse.bass as bass
import concourse.mybir as mybir
import concourse.tile as tile
from concourse import bacc
from concourse import bass_utils

ts = bass.ts
F32 = mybir.dt.float32
F16 = mybir.dt.float16
F8 = mybir.dt.float8e4
F8E5 = mybir.dt.float8e5
U8 = mybir.dt.uint8
DRM = mybir.MatmulPerfMode.DoubleRow

P = 128          # SBUF partitions
N = 2048         # sequence length
D = 1024         # model dim
DLOC = 512       # local (per-core) feature dim = 8 heads * 64
DH = 64          # head dim
VW = 80          # padded V' width: 64 V + 1 ones + 15 zero (16B-aligned)
NPAIR = 4        # head pairs per core (2 heads per pair = 128 feats)
NCO = D // P     # 8 fp16 contraction tiles over model dim
NDR = NCO // 2   # 4 DoubleRow contraction tiles
NJT = N // P     # 16 key tiles of 128
NCH = N // 512   # 4 query chunks of 512
SCALE = DH ** -0.5
EXPBIAS = -3.0   # exp(s*SCALE + EXPBIAS): keeps e^s inside e4m3 range

# Schraudolph exp on DVE (e5m2 bits = 4*(log2e*(s*SCALE+EXPBIAS) + 15)):
# full j-tile pairs with jp % DVE_MOD == 1 route their exp to DVE to
# offload ScalarE.  0 disables.
DVE_MOD = 3
LOG2E = float(np.log2(np.e))
SCH_SHIFT = 0.04367  # mean-centers the (1+f)/2^f mantissa-interp error
SCH_A = 4.0 * LOG2E * SCALE
SCH_B = 4.0 * (15.0 + LOG2E * EXPBIAS - SCH_SHIFT)


def _emit_kernel(tc, xT0_d, x8_d, wq16_d, wq8_d, wk16_d, wk8_d, wv16_d,
                 wv8_d, wo_d, bq_d, bk_d, out_d):
    nc = tc.nc
    EXP = mybir.ActivationFunctionType.Exp
    GE = mybir.AluOpType.is_ge

    xT0r = xT0_d.rearrange("(o p) t -> p o t", p=P)
    wq16r = wq16_d.rearrange("(o p) f -> p o f", p=P)
    wk16r = wk16_d.rearrange("(o p) f -> p o f", p=P)
    wv16r = wv16_d.rearrange("(o p) f -> p o f", p=P)
    wor = wo_d.rearrange("(o p) e -> p o e", p=P)

    with (
        nc.allow_low_precision(reason="fp8/fp16 operands, fp32 accumulation"),
        tc.tile_pool(name="perm", bufs=1) as perm,
        tc.tile_pool(name="shared", bufs=1) as shared,
        tc.tile_pool(name="qkt", bufs=3) as qktp,
        tc.tile_pool(name="pexp", bufs=7) as pexp,
        tc.tile_pool(name="stg", bufs=4) as stgp,
        tc.tile_pool(name="rrp", bufs=3) as rrp,
        tc.tile_pool(name="outsb", bufs=3) as outsb,
        tc.tile_pool(name="psS", bufs=2, space="PSUM") as psS,
        tc.tile_pool(name="psO", bufs=1, space="PSUM") as psO,
        tc.tile_pool(name="psProj", bufs=2, space="PSUM") as psProj,
    ):
        # ---- PE warmup: span the HAM activity window while DMAs land ----
        dummy = perm.tile([P, 256], F16, name="dummy")
        nc.vector.memset(dummy, 0.0)
        dps = psProj.tile([P, 512], F32, name="dps", tag="proj")
        for _ in range(16):
            nc.tensor.matmul(
                dps[:, 0:256], lhsT=dummy[:, 0:P], rhs=dummy, start=True, stop=True
            )

        # ---- constants / weights ----------------------------------------
        bq_sb = perm.tile([P, NPAIR], F32, name="bq_sb")
        bk_sb = perm.tile([P, NPAIR], F32, name="bk_sb")
        ebias = perm.tile([P, 1], F32, name="ebias")
        nc.vector.memset(ebias, EXPBIAS)
        # V'16 = [V | 1] (j-tiles 0-3, chunk-0 PV); V'8 = [V | 1 | 0-pad]
        # (all j-tiles, DR PV: ones column accumulates softmax denominators,
        # zero pad reaches the 16B-aligned width DR weights require)
        Vp16 = perm.tile([P, 4, 8, DH + 1], F16, name="Vp16")
        nc.vector.memset(Vp16[:, :, :, DH:], 1.0)
        Vp8 = perm.tile([P, NJT // 2, 2, 8, VW], F8, name="Vp8")
        nc.vector.memset(Vp8[:, :, :, :, DH:], 0.0)
        nc.vector.memset(Vp8[:, :, :, :, DH : DH + 1], 1.0)

        xT16 = perm.tile([P, NCO, 512], F16, name="xT16")
        xT8 = perm.tile([P, NDR, 2, N - 512], F8, name="xT8")
        wq16_sb = shared.tile([P, NCO, DLOC], F16, name="wq16_sb", tag="wq_wo")
        wk16_sb = shared.tile([P, NCO, DLOC], F16, name="wk16_sb", tag="wk")
        wv16_sb = shared.tile([P, NCO, DLOC], F16, name="wv16_sb", tag="wv")
        wq8_sb = perm.tile([P, NDR, 2, DLOC], F8, name="wq8_sb")
        wk8_sb = perm.tile([P, NDR, 2, DLOC], F8, name="wk8_sb")
        wv8_sb = perm.tile([P, NDR, 2, DLOC], F8, name="wv8_sb")
        # Critical-path DMAs first, spread over the three DMA-capable
        # engine queues, within-queue order matching first-use time:
        # (1) pair-0 Wq/Wk fp16 slices + xT16 per-co pieces (chunk-0 proj,
        # t~3us), (2) wv16 per-co pieces (V j-tiles 0-3, t~10us), (3) fp8
        # weights + xT8 first chunk (pair-0 ch1 fill), (4) remaining fp16
        # Wq/Wk columns (pair-1 ch0 proj, t~55us), (5) xT8 tail chunks.
        engs = (nc.sync, nc.scalar, nc.gpsimd)
        nc.gpsimd.dma_start(out=bq_sb, in_=bq_d)
        nc.gpsimd.dma_start(out=bk_sb, in_=bk_d)
        for co in range(NCO):
            eng = engs[co % 3]
            eng.dma_start(out=wq16_sb[:, co, 0:P], in_=wq16r[:, co, 0:P])
            engs[(co + 1) % 3].dma_start(
                out=wk16_sb[:, co, 0:P], in_=wk16r[:, co, 0:P]
            )
            eng.dma_start(out=xT16[:, co, :], in_=xT0r[:, co, :])
        for co in range(NCO):
            engs[co % 3].dma_start(out=wv16_sb[:, co, :], in_=wv16r[:, co, :])
        nc.sync.dma_start(out=wq8_sb.bitcast(U8), in_=wq8_d)
        nc.scalar.dma_start(out=wk8_sb.bitcast(U8), in_=wk8_d)
        nc.gpsimd.dma_start(out=xT8[:, :, :, 0:512].bitcast(U8),
                            in_=x8_d[:, :, :, 0:512])
        nc.sync.dma_start(out=wv8_sb.bitcast(U8), in_=wv8_d)
        nc.scalar.dma_start(out=wq16_sb[:, :, P:DLOC], in_=wq16r[:, :, P:DLOC])
        nc.gpsimd.dma_start(out=xT8[:, :, :, 512:1024].bitcast(U8),
                            in_=x8_d[:, :, :, 512:1024])
        nc.sync.dma_start(out=wk16_sb[:, :, P:DLOC], in_=wk16r[:, :, P:DLOC])
        nc.scalar.dma_start(out=xT8[:, :, :, 1024:1536].bitcast(U8),
                            in_=x8_d[:, :, :, 1024:1536])

        qk_tiles = {}

        def qkproj_gen(pair, use_big_psum, chunks=None):
            """Q^T / K^T projection for one head pair; yields between ops.

            Chunk 0 is the fp16 path (8 co-tile matmuls); chunks 1-3 run
            fp8 DoubleRow (4 dr-tile matmuls).  PSUM evictions (+bias) go
            to DVE: ScalarE must stay exp-only during attention phases.
            """
            if pair not in qk_tiles:
                qk_tiles[pair] = (
                    qktp.tile([P, N], F16, name=f"QT{pair}", tag="qt"),
                    qktp.tile([P, N], F16, name=f"KT{pair}", tag="kt"),
                )
            QT, KT = qk_tiles[pair]
            for ch in chunks if chunks is not None else range(NCH):
                for w16, w8, dst, bias in (
                    (wq16_sb, wq8_sb, QT, bq_sb),
                    (wk16_sb, wk8_sb, KT, bk_sb),
                ):
                    if use_big_psum:
                        grp = psS.tile([P, 2, 512], F32, name="pj", tag="sg")
                        acc = grp[:, 0, :]
                    else:
                        acc = psProj.tile([P, 512], F32, name="pj", tag="proj")
                    if ch == 0:
                        for co in range(NCO):
                            nc.tensor.matmul(
                                acc,
                                lhsT=w16[:, co, ts(pair, P)],
                                rhs=xT16[:, co, :],
                                start=(co == 0),
                                stop=(co == NCO - 1),
                            )
                            yield
                    else:
                        for cp in range(NDR):
                            nc.tensor.matmul(
                                acc,
                                lhsT=w8[:, cp, :, ts(pair, P)],
                                rhs=xT8[:, cp, :, ts(ch - 1, 512)],
                                start=(cp == 0),
                                stop=(cp == NDR - 1),
                                perf_mode=DRM,
                            )
                            yield
                    nc.vector.tensor_scalar_add(
                        out=dst[:, ts(ch, 512)],
                        in0=acc,
                        scalar1=bias[:, pair : pair + 1],
                    )
                    yield "end"

        def vproj_gen(jt0, jt1, use_big_psum):
            for jt in range(jt0, jt1):
                if use_big_psum:
                    grp = psS.tile([P, 2, 512], F32, name="vps", tag="sg")
                    acc = grp[:, 0, :]
                else:
                    acc = psProj.tile([P, 512], F32, name="vps", tag="proj")
                if jt < 4:
                    for co in range(NCO):
                        nc.tensor.matmul(
                            acc,
                            lhsT=xT16[:, co, ts(jt, P)],
                            rhs=wv16_sb[:, co, :],
                            start=(co == 0),
                            stop=(co == NCO - 1),
                        )
                        yield
                    nc.vector.tensor_copy(
                        out=Vp16[:, jt, :, 0:DH],
                        in_=acc.rearrange("p (h f) -> p h f", h=8),
                    )
                else:
                    for cp in range(NDR):
                        nc.tensor.matmul(
                            acc,
                            lhsT=xT8[:, cp, :, ts(jt - 4, P)],
                            rhs=wv8_sb[:, cp, :, :],
                            start=(cp == 0),
                            stop=(cp == NDR - 1),
                            perf_mode=DRM,
                        )
                        yield
                nc.vector.tensor_copy(
                    out=Vp8[:, jt // 2, jt % 2, :, 0:DH],
                    in_=acc.rearrange("p (h f) -> p h f", h=8),
                )
                yield "end"

        def chain(*gens):
            for g in gens:
                yield from g

        class Fill:
            """Dispenses filler ops; a PSUM-accumulator group must never
            straddle an attention chunk boundary (its DVE eviction would
            queue behind the next chunk's PV start while PV waits on the
            slot that eviction frees -> deadlock)."""

            def __init__(self, gen, count0=0):
                self.gen = gen
                self.in_group = False
                self.count = count0

            def _next(self):
                v = next(self.gen, StopIteration)
                if v is StopIteration:
                    self.gen = None
                    self.in_group = False
                    return False
                self.count += 1
                self.in_group = v != "end"
                return True

            def append_gen(self, g):
                self.gen = chain(self.gen, g) if self.gen is not None else g

            def pull(self, n):
                for _ in range(n):
                    if self.gen is None or not self._next():
                        return

            def pull_to(self, target):
                while self.count < target and self.gen is not None:
                    if not self._next():
                        return

            def finish_group(self):
                while self.gen is not None and self.in_group:
                    self._next()

            def drain(self):
                while self.gen is not None and self._next():
                    pass

        def attn_chunk0(pair, fill, fill_rate, pre_pv=None):
            """Chunk 0, all-fp16 (short causal rows need the precision)."""
            QT, KT = qk_tiles[pair]
            hA, hB = 2 * pair, 2 * pair + 1
            oA = psO.tile([P, 512], F32, name="oA", tag="oA")
            oB = psO.tile([P, 512], F32, name="oB", tag="oB")
            pend = []

            def pv1(pt, jt, oP, h, h01):
                plo = P * jt if jt > 0 else 0
                nc.tensor.matmul(
                    oP[0 : DH + 1, plo:512],
                    lhsT=Vp16[:, jt, h, :],
                    rhs=pt[:, h01, plo:512],
                    start=(jt == 0),
                    stop=(jt == 3),
                )

            def pv_pair(a, b):
                for oP, h, h01 in ((oA, hA, 0), (oB, hB, 1)):
                    pv1(a[0], a[1], oP, h, h01)
                    pv1(b[0], b[1], oP, h, h01)

            for jt2 in range(0, 4, 2):
                if fill is not None:
                    fill.pull(fill_rate)
                for jt in (jt2, jt2 + 1):
                    sg = psS.tile([P, 2, 512], F32, name="sg", tag="sg")
                    lo = P * jt if jt > 0 else 0
                    nc.tensor.matmul(
                        sg[:, 0, lo:512],
                        lhsT=KT[0:DH, ts(jt, P)],
                        rhs=QT[0:DH, lo:512],
                        start=True,
                        stop=True,
                    )
                    nc.tensor.matmul(
                        sg[:, 1, lo:512],
                        lhsT=KT[DH:P, ts(jt, P)],
                        rhs=QT[DH:P, lo:512],
                        start=True,
                        stop=True,
                    )
                    pt = pexp.tile([P, 2, 512], F16, name="pt", tag="pt")
                    nc.scalar.activation(
                        out=pt[:, :, lo:512], in_=sg[:, :, lo:512], func=EXP,
                        scale=SCALE, bias=ebias,
                    )
                    nc.gpsimd.affine_select(
                        out=pt[:, :, lo : lo + P],
                        in_=pt[:, :, lo : lo + P],
                        compare_op=GE,
                        fill=0.0,
                        base=0,
                        channel_multiplier=-1,
                        pattern=[[0, 2], [1, P]],
                    )
                    pend.append((pt, jt))
            if pre_pv is not None:
                pre_pv()
            while pend:
                a = pend.pop(0)
                b = pend.pop(0)
                pv_pair(a, b)
            if fill is not None:
                fill.finish_group()
            return oA, oB

        def attn_chunk_dr(pair, ch, fill, fill_rate):
            """Chunks 1-3: fp16 S -> e4m3 pt -> DoubleRow PV over j-tile
            pairs.  Diagonal pairs: (r0,r1) full width with r1's cols
            [0:128] affine-zeroed; (r2,r3) cols [256:512] with r3's
            [256:384] affine-zeroed."""
            QT, KT = qk_tiles[pair]
            hA, hB = 2 * pair, 2 * pair + 1
            oA = psO.tile([P, 512], F32, name="oA", tag="oA")
            oB = psO.tile([P, 512], F32, name="oB", tag="oB")
            njp = 2 * ch + 2
            pend = []

            def dr_pv1(pt, jp, h01):
                plo = 256 if jp == njp - 1 else 0
                oP, h = ((oA, hA), (oB, hB))[h01]
                nc.tensor.matmul(
                    oP[0:VW, plo:512],
                    lhsT=Vp8[:, jp, :, h, :],
                    rhs=pt[:, :, h01, plo:512],
                    start=(jp == 0),
                    stop=(jp == njp - 1),
                    perf_mode=DRM,
                )

            for jp in range(njp):
                if fill is not None:
                    fill.pull(fill_rate)
                # full pairs on the DVE cadence use the Schraudolph exp
                # (one tensor_scalar producing e5m2 bits) to offload ScalarE
                dve = DVE_MOD > 0 and jp < 2 * ch and jp % DVE_MOD == 1
                pt = pexp.tile([P, 2, 2, 512], F8E5 if dve else F8,
                               name="pt", tag="pt")
                for jh in (0, 1):
                    jt = 2 * jp + jh
                    r = jt - 4 * ch
                    lo = P * r if r > 0 else 0
                    sg = psS.tile([P, 2, 512], F32, name="sg", tag="sg")
                    nc.tensor.matmul(
                        sg[:, 0, lo:512],
                        lhsT=KT[0:DH, ts(jt, P)],
                        rhs=QT[0:DH, 512 * ch + lo : 512 * (ch + 1)],
                        start=True,
                        stop=True,
                    )
                    nc.tensor.matmul(
                        sg[:, 1, lo:512],
                        lhsT=KT[DH:P, ts(jt, P)],
                        rhs=QT[DH:P, 512 * ch + lo : 512 * (ch + 1)],
                        start=True,
                        stop=True,
                    )
                    if dve:
                        nc.vector.tensor_scalar(
                            out=pt[:, jh, :, :].bitcast(U8), in0=sg,
                            scalar1=SCH_A, scalar2=SCH_B,
                            op0=mybir.AluOpType.mult,
                            op1=mybir.AluOpType.add,
                        )
                    else:
                        nc.scalar.activation(
                            out=pt[:, jh, :, lo:512], in_=sg[:, :, lo:512],
                            func=EXP, scale=SCALE, bias=ebias,
                        )
                    if r >= 0:
                        # keep where q >= 128*r + p; odd-r tiles are the
                        # second half of a DR pair whose matmul starts 128
                        # cols earlier -- zero that uncomputed strip
                        nc.gpsimd.affine_select(
                            out=pt[:, jh, :, lo : lo + P],
                            in_=pt[:, jh, :, lo : lo + P],
                            compare_op=GE,
                            fill=0.0,
                            base=0,
                            channel_multiplier=-1,
                            pattern=[[0, 2], [1, P]],
                        )
                        if r % 2 == 1:
                            nc.gpsimd.memset(pt[:, jh, :, lo - P : lo], 0.0)
                    if pend:
                        dr_pv1(*pend[0], jh)
                        if jh == 1:
                            pend.pop(0)
                pend.append((pt, jp))
            while pend:
                a = pend.pop(0)
                dr_pv1(*a, 0)
                dr_pv1(*a, 1)
            if fill is not None:
                fill.finish_group()
            return oA, oB

        def attn_emit(pair, fill, after_chunk=None, ascending=False,
                      pre_chunk=None, fill_rate=4, pre_pv=None):
            chunk_order = (
                list(range(NCH)) if ascending else list(range(NCH - 1, -1, -1))
            )
            for chi, ch in enumerate(chunk_order):
                if pre_chunk is not None:
                    pre_chunk(ch)
                if after_chunk is not None and chi > 0:
                    after_chunk(chunk_order[chi - 1])
                if ch == 0:
                    oA, oB = attn_chunk0(pair, fill, fill_rate, pre_pv=pre_pv)
                else:
                    oA, oB = attn_chunk_dr(pair, ch, fill, fill_rate)

                # ---- stage O' out of PSUM, then normalize off-path --------
                st = stgp.tile([DH + 1, 2, 512], F16, name="st", tag="st")
                # split staging across ScalarE+DVE: the PSUM banks (and the
                # next chunk's PV) unblock in half the time
                nc.scalar.copy(out=st[:, 0, :], in_=oA[0 : DH + 1, :])
                nc.vector.tensor_copy(out=st[:, 1, :], in_=oB[0 : DH + 1, :])
                # Reciprocal of the 1024 sums at full DVE lane utilization:
                # DMA-repack the denominator row onto 128 partitions, recip
                # there, unpack to a partition-0 row, then gpsimd-broadcast
                # it down 64 partitions.
                pk = rrp.tile([P, 8], F16, name="pk", tag="pk")
                rrow = rrp.tile([1, 2, 512], F16, name="rrow", tag="rrow")
                Rs = rrp.tile([DH, 2, 512], F16, name="Rs", tag="Rs")
                nc.sync.dma_start(out=pk, in_=st[DH : DH + 1, :, :])
                nc.vector.reciprocal(out=pk, in_=pk)
                nc.gpsimd.dma_start(out=rrow, in_=pk)
                nc.gpsimd.partition_broadcast(Rs, rrow[0:1, :, :])
                nc.vector.tensor_mul(
                    out=OT[0:DH, pair, ts(ch, 512)], in0=st[0:DH, 0, :],
                    in1=Rs[:, 0, :],
                )
                nc.vector.tensor_mul(
                    out=OT[DH:P, pair, ts(ch, 512)], in0=st[0:DH, 1, :],
                    in1=Rs[:, 1, :],
                )
                if after_chunk is not None and chi == NCH - 1:
                    after_chunk(ch)
            if fill is not None:
                fill.drain()

        def outproj_gen(ch, tail=False):
            for it in range(4 * ch, 4 * ch + 4):
                ob = outsb.tile([P, 2, 512], F16, name="ob", tag="ob")
                for e in range(2):
                    acc = psProj.tile([P, 512], F32, name="ops", tag="proj")
                    for p4 in range(NPAIR):
                        nc.tensor.matmul(
                            acc,
                            lhsT=OT[:, p4, ts(it, P)],
                            rhs=wo_sb[:, p4, ts(e, 512)],
                            start=(p4 == 0),
                            stop=(p4 == NPAIR - 1),
                        )
                        yield
                    # the tail chunk evicts on ScalarE (exp is done by then);
                    # mid-attention chunks use DVE (gpsimd can't read PSUM)
                    if tail and e == 0:
                        nc.scalar.copy(out=ob[:, e, :], in_=acc)
                    else:
                        nc.vector.tensor_copy(out=ob[:, e, :], in_=acc)
                    if e == 1:
                        nc.sync.dma_start(out=out_d[ts(it, P), :], in_=ob)
                    yield "end"

        # ---- emission schedule ------------------------------------------
        # upfront: only chunk 0 of pair-0's Q/K (fp16) and the first 4 V
        # j-tiles (fp16); pair-0 attention then runs ascending-chunk with a
        # fill chain of per-chunk blocks [Q ch, K ch, V 4ch..4ch+4] --
        # chunk ch only needs Q/K chunks <= ch and V j-tiles < 4ch+4,
        # guaranteed emitted by pull_to before the chunk starts -- followed
        # by pair-1's projections.
        for _ in qkproj_gen(0, use_big_psum=True, chunks=[0]):
            pass
        for _ in vproj_gen(0, 4, use_big_psum=True):
            pass

        OT = shared.tile([P, NPAIR, N], F16, name="OT", tag="ot")

        def primed(gen):
            next(gen)
            return gen

        def pair0_fill_gen():
            for ch in range(1, NCH):
                yield from qkproj_gen(0, False, chunks=[ch])
                yield from vproj_gen(4 * ch, 4 * ch + 4, False)
            yield from qkproj_gen(1, False)

        # fill-block op counts: vproj(0,4) = 36, then per-chunk blocks of
        # 30 (DR qk = 10 + DR vproj = 20)
        fill0 = Fill(primed(pair0_fill_gen()), count0=1)
        attn_emit(
            0,
            fill0,
            ascending=True,
            pre_chunk=lambda ch: fill0.pull_to(30 * ch),
            fill_rate=8,
        )
        attn_emit(1, Fill(primed(qkproj_gen(2, use_big_psum=False)), count0=1))
        attn_emit(2, Fill(primed(qkproj_gen(3, use_big_psum=False)), count0=1))

        # wo reuses wq16's slot (wq16 dead after pair-3 projections)
        wo_sb = shared.tile([P, NPAIR, D], F16, name="wo_sb", tag="wq_wo")
        for o4 in range(NPAIR):
            nc.sync.dma_start(out=wo_sb[:, o4, :], in_=wor[:, o4, :])

        # pair 3 with the output projection streaming through its fill
        # slots: each finished (normalized) chunk's outproj ops are appended
        # to the fill and drawn between S/PV groups
        fill3 = Fill(iter(()))
        attn_emit(
            3, fill3,
            after_chunk=lambda ch: fill3.append_gen(
                outproj_gen(ch, tail=(ch == 0))
            ),
            fill_rate=8,
        )


def build():
    nc = bacc.Bacc("TRN2", target_bir_lowering=False, debug=False, num_devices=8)
    xT0_d = nc.dram_tensor("xT0", [D, 512], F16, kind="ExternalInput").ap()
    x8_d = nc.dram_tensor("x8", [P, NDR, 2, N - 512], U8, kind="ExternalInput").ap()
    wq16_d = nc.dram_tensor("wq16", [D, DLOC], F16, kind="ExternalInput").ap()
    wq8_d = nc.dram_tensor("wq8", [P, NDR, 2, DLOC], U8, kind="ExternalInput").ap()
    wk16_d = nc.dram_tensor("wk16", [D, DLOC], F16, kind="ExternalInput").ap()
    wk8_d = nc.dram_tensor("wk8", [P, NDR, 2, DLOC], U8, kind="ExternalInput").ap()
    wv16_d = nc.dram_tensor("wv16", [D, DLOC], F16, kind="ExternalInput").ap()
    wv8_d = nc.dram_tensor("wv8", [P, NDR, 2, DLOC], U8, kind="ExternalInput").ap()
    wo_d = nc.dram_tensor("wo", [DLOC, D], F16, kind="ExternalInput").ap()
    bq_d = nc.dram_tensor("bq", [P, NPAIR], F32, kind="ExternalInput").ap()
    bk_d = nc.dram_tensor("bk", [P, NPAIR], F32, kind="ExternalInput").ap()
    out_d = nc.dram_tensor("out", [N, D], F16, kind="ExternalOutput").ap()
    with tile.TileContext(nc) as tc:
        _emit_kernel(tc, xT0_d, x8_d, wq16_d, wq8_d, wk16_d, wk8_d, wv16_d,
                     wv8_d, wo_d, bq_d, bk_d, out_d)
    nc.compile()
    return nc


_NC = None


def _get_nc():
    global _NC
    if _NC is None:
        _NC = build()
    return _NC


def _to_dr8(w):
    """[1024, F] fp32 -> [128, 4, 2, F] e4m3 bits (d = (2*cp + i)*128 + p)."""
    f = w.shape[1]
    return np.ascontiguousarray(
        w.reshape(NDR, 2, P, f).transpose(2, 0, 1, 3)
        .astype(ml_dtypes.float8_e4m3).view(np.uint8)
    )


def make_in_maps(x, Wq, bq, Wkv, bkv, Wo, bo):
    x = np.asarray(x, dtype=np.float32)
    Wq = np.asarray(Wq, dtype=np.float32)
    bq = np.asarray(bq, dtype=np.float32)
    Wkv = np.asarray(Wkv, dtype=np.float32)
    bkv = np.asarray(bkv, dtype=np.float32)
    Wo = np.asarray(Wo, dtype=np.float32)

    in_maps = []
    for c in range(8):
        bi, hj = c // 2, c % 2
        sl = slice(hj * DLOC, (hj + 1) * DLOC)
        slv = slice(D + hj * DLOC, D + (hj + 1) * DLOC)
        xT = np.ascontiguousarray(x[bi].T)
        in_maps.append(
            {
                "xT0": xT[:, 0:512].astype(np.float16),
                "x8": _to_dr8(xT[:, 512:]),
                "wq16": np.ascontiguousarray(Wq[:, sl]).astype(np.float16),
                "wq8": _to_dr8(Wq[:, sl]),
                "wk16": np.ascontiguousarray(Wkv[:, sl]).astype(np.float16),
                "wk8": _to_dr8(Wkv[:, sl]),
                "wv16": np.ascontiguousarray(Wkv[:, slv]).astype(np.float16),
                "wv8": _to_dr8(Wkv[:, slv]),
                "wo": np.ascontiguousarray(Wo[sl, :]).astype(np.float16),
                "bq": np.ascontiguousarray(bq[sl].reshape(NPAIR, P).T),
                "bk": np.ascontiguousarray(bkv[sl].reshape(NPAIR, P).T),
            }
        )
    return in_maps


def combine_outputs(results, bkv, Wo, bo):
    bo = np.asarray(bo, dtype=np.float32)
    bv = np.asarray(bkv, dtype=np.float32)[D:]
    c = bv @ np.asarray(Wo, dtype=np.float32) + bo
    outs = [results[cid]["out"].astype(np.float32) for cid in range(8)]
    full = np.stack([outs[2 * bi] + outs[2 * bi + 1] for bi in range(4)])
    return (full + c[None, None, :]).astype(np.float32)


def kernel(x, Wq, bq, Wkv, bkv, Wo, bo, **_ignored):
    nc = _get_nc()
    in_maps = make_in_maps(x, Wq, bq, Wkv, bkv, Wo, bo)
    res = bass_utils.run_bass_kernel_spmd(nc, in_maps, core_ids=list(range(8)))
    return combine_outputs(res.results, bkv, Wo, bo)


# revision 27
# speedup vs baseline: 1.1121x; 1.1121x over previous
"""Trainium2 Bass kernel for causal multi-head attention (fp8 DoubleRow).

Problem: x[4, 2048, 1024] -> Attention(heads=16, causal) -> out[4, 2048, 1024]

Sharding over 8 NeuronCores: core c handles batch bi = c // 2 and head-half
hj = c % 2 (8 of the 16 heads).  Each core computes its 8 heads' attention
and a partial output projection (row-parallel Wo); the host sums the two
partials per batch element and adds bv@Wo + bo.

Numerical split (tolerance is 2e-2 of max|y|; short causal rows are the
error-sensitive ones, long rows average fp8 noise away):
  - Tokens 0-511 everywhere they can feed a short row stay fp16: Q/K/V
    projections for chunk 0 / j-tiles 0-3, chunk-0 S, chunk-0 PV.
  - Tokens 512+ projections run fp8(e4m3) DoubleRow (2 contraction tiles
    per instruction -> 2x PE throughput).
  - Chunks 1-3 PV runs fp8 DoubleRow pairing two j-tiles per matmul;
    pt = exp(s/8 - 3) written by ScalarE directly in e4m3 (bias -3 keeps
    e^s <= ~90 < 240), V' = [V | 1 | 0-pad] padded to 80 cols (DR weight
    free-dim must be 16B-aligned; DR output must start at partition 0).
    The uncomputed below-diagonal strip of each DR pair's second tile is
    gpsimd-memset to 0.
  - One in three full j-tile pairs routes its exp to DVE as a Schraudolph
    bit-trick (one tensor_scalar producing e5m2 bits; negative bits only
    at |s| > 7 sigma, where the fp32->uint8 conversion saturates to 0),
    offloading ScalarE, the attention-phase pace-setter.
  - S stays fp16 (K=64 row-packed pairs already run at ~1 col/cycle for
    both heads; fp8 adds nothing), out-projection stays fp16 (fp8 there
    fails the error budget).

The emission schedule (fill architecture) is inherited from the fp16
kernel: projection work streams through attention fill slots so the PE
never starves while ScalarE catches up on exp.  Engine budget per core:
PE ~205us busy, ScalarE ~120us (exp), DVE ~125us (evictions, staging,
normalize, Schraudolph), GpSimd ~80us (affine masks, broadcasts).
"""

import os
import sys

for _p in ("/opt/trn_rl_repo",):
    if _p not in sys.path and os.path.isdir(_p):
        sys.path.insert(0, _p)

import numpy as np
import ml_dtypes

import concourse.bass as bass
import concourse.mybir as mybir
import concourse.tile as tile
from concourse import bacc
from concourse import bass_utils

ts = bass.ts
F32 = mybir.dt.float32
F16 = mybir.dt.float16
F8 = mybir.dt.float8e4
F8E5 = mybir.dt.float8e5
U8 = mybir.dt.uint8
DRM = mybir.MatmulPerfMode.DoubleRow

P = 128          # SBUF partitions
N = 2048         # sequence length
D = 1024         # model dim
DLOC = 512       # local (per-core) feature dim = 8 heads * 64
DH = 64          # head dim
VW = 80          # padded V' width: 64 V + 1 ones + 15 zero (16B-aligned)
NPAIR = 4        # head pairs per core (2 heads per pair = 128 feats)
NCO = D // P     # 8 fp16 contraction tiles over model dim
NDR = NCO // 2   # 4 DoubleRow contraction tiles
NJT = N // P     # 16 key tiles of 128
NCH = N // 512   # 4 query chunks of 512
SCALE = DH ** -0.5
EXPBIAS = -3.0   # exp(s*SCALE + EXPBIAS): keeps e^s inside e4m3 range

# Schraudolph exp on DVE (e5m2 bits = 4*(log2e*(s*SCALE+EXPBIAS) + 15)):
# full j-tile pairs with jp % DVE_MOD == 1 route their exp to DVE to
# offload ScalarE.  0 disables.
DVE_MOD = 3
LOG2E = float(np.log2(np.e))
SCH_SHIFT = 0.04367  # mean-centers the (1+f)/2^f mantissa-interp error
SCH_A = 4.0 * LOG2E * SCALE
SCH_B = 4.0 * (15.0 + LOG2E * EXPBIAS - SCH_SHIFT)


def _emit_kernel(tc, xT0_d, x8_d, wq16_d, wq8_d, wk16_d, wk8_d, wv16_d,
                 wv8_d, wo_d, bq_d, bk_d, out_d):
    nc = tc.nc
    EXP = mybir.ActivationFunctionType.Exp
    GE = mybir.AluOpType.is_ge

    xT0r = xT0_d.rearrange("(o p) t -> p o t", p=P)
    wq16r = wq16_d.rearrange("(o p) f -> p o f", p=P)
    wk16r = wk16_d.rearrange("(o p) f -> p o f", p=P)
    wv16r = wv16_d.rearrange("(o p) f -> p o f", p=P)
    wor = wo_d.rearrange("(o p) e -> p o e", p=P)

    with (
        nc.allow_low_precision(reason="fp8/fp16 operands, fp32 accumulation"),
        tc.tile_pool(name="perm", bufs=1) as perm,
        tc.tile_pool(name="shared", bufs=1) as shared,
        tc.tile_pool(name="qkt", bufs=3) as qktp,
        tc.tile_pool(name="pexp", bufs=7) as pexp,
        tc.tile_pool(name="stg", bufs=4) as stgp,
        tc.tile_pool(name="rrp", bufs=3) as rrp,
        tc.tile_pool(name="outsb", bufs=3) as outsb,
        tc.tile_pool(name="psS", bufs=2, space="PSUM") as psS,
        tc.tile_pool(name="psO", bufs=1, space="PSUM") as psO,
        tc.tile_pool(name="psProj", bufs=2, space="PSUM") as psProj,
    ):
        # ---- PE warmup: span the HAM activity window while DMAs land ----
        dummy = perm.tile([P, 256], F16, name="dummy")
        nc.vector.memset(dummy, 0.0)
        dps = psProj.tile([P, 512], F32, name="dps", tag="proj")
        for _ in range(16):
            nc.tensor.matmul(
                dps[:, 0:256], lhsT=dummy[:, 0:P], rhs=dummy, start=True, stop=True
            )

        # ---- constants / weights ----------------------------------------
        bq_sb = perm.tile([P, NPAIR], F32, name="bq_sb")
        bk_sb = perm.tile([P, NPAIR], F32, name="bk_sb")
        ebias = perm.tile([P, 1], F32, name="ebias")
        nc.vector.memset(ebias, EXPBIAS)
        # V'16 = [V | 1] (j-tiles 0-3, chunk-0 PV); V'8 = [V | 1 | 0-pad]
        # (all j-tiles, DR PV: ones column accumulates softmax denominators,
        # zero pad reaches the 16B-aligned width DR weights require)
        Vp16 = perm.tile([P, 4, 8, DH + 1], F16, name="Vp16")
        nc.vector.memset(Vp16[:, :, :, DH:], 1.0)
        Vp8 = perm.tile([P, NJT // 2, 2, 8, VW], F8, name="Vp8")
        nc.vector.memset(Vp8[:, :, :, :, DH:], 0.0)
        nc.vector.memset(Vp8[:, :, :, :, DH : DH + 1], 1.0)

        xT16 = perm.tile([P, NCO, 512], F16, name="xT16")
        xT8 = perm.tile([P, NDR, 2, N - 512], F8, name="xT8")
        wq16_sb = shared.tile([P, NCO, DLOC], F16, name="wq16_sb", tag="wq_wo")
        wk16_sb = shared.tile([P, NCO, DLOC], F16, name="wk16_sb", tag="wk")
        wv16_sb = shared.tile([P, NCO, DLOC], F16, name="wv16_sb", tag="wv")
        wq8_sb = perm.tile([P, NDR, 2, DLOC], F8, name="wq8_sb")
        wk8_sb = perm.tile([P, NDR, 2, DLOC], F8, name="wk8_sb")
        wv8_sb = perm.tile([P, NDR, 2, DLOC], F8, name="wv8_sb")
        # Critical-path DMAs first, spread over the three DMA-capable
        # engine queues, within-queue order matching first-use time:
        # (1) pair-0 Wq/Wk fp16 slices + xT16 per-co pieces (chunk-0 proj,
        # t~3us), (2) wv16 per-co pieces (V j-tiles 0-3, t~10us), (3) fp8
        # weights + xT8 first chunk (pair-0 ch1 fill), (4) remaining fp16
        # Wq/Wk columns (pair-1 ch0 proj, t~55us), (5) xT8 tail chunks.
        engs = (nc.sync, nc.scalar, nc.gpsimd)
        nc.gpsimd.dma_start(out=bq_sb, in_=bq_d)
        nc.gpsimd.dma_start(out=bk_sb, in_=bk_d)
        for co in range(NCO):
            eng = engs[co % 3]
            eng.dma_start(out=wq16_sb[:, co, 0:P], in_=wq16r[:, co, 0:P])
            engs[(co + 1) % 3].dma_start(
                out=wk16_sb[:, co, 0:P], in_=wk16r[:, co, 0:P]
            )
            eng.dma_start(out=xT16[:, co, :], in_=xT0r[:, co, :])
        for co in range(NCO):
            engs[co % 3].dma_start(out=wv16_sb[:, co, :], in_=wv16r[:, co, :])
        nc.sync.dma_start(out=wq8_sb.bitcast(U8), in_=wq8_d)
        nc.scalar.dma_start(out=wk8_sb.bitcast(U8), in_=wk8_d)
        nc.gpsimd.dma_start(out=xT8[:, :, :, 0:512].bitcast(U8),
                            in_=x8_d[:, :, :, 0:512])
        nc.sync.dma_start(out=wv8_sb.bitcast(U8), in_=wv8_d)
        nc.scalar.dma_start(out=wq16_sb[:, :, P:DLOC], in_=wq16r[:, :, P:DLOC])
        nc.gpsimd.dma_start(out=xT8[:, :, :, 512:1024].bitcast(U8),
                            in_=x8_d[:, :, :, 512:1024])
        nc.sync.dma_start(out=wk16_sb[:, :, P:DLOC], in_=wk16r[:, :, P:DLOC])
        nc.scalar.dma_start(out=xT8[:, :, :, 1024:1536].bitcast(U8),
                            in_=x8_d[:, :, :, 1024:1536])

        qk_tiles = {}

        def qkproj_gen(pair, use_big_psum, chunks=None):
            """Q^T / K^T projection for one head pair; yields between ops.

            Chunk 0 is the fp16 path (8 co-tile matmuls); chunks 1-3 run
            fp8 DoubleRow (4 dr-tile matmuls).  PSUM evictions (+bias) go
            to DVE: ScalarE must stay exp-only during attention phases.
            """
            if pair not in qk_tiles:
                qk_tiles[pair] = (
                    qktp.tile([P, N], F16, name=f"QT{pair}", tag="qt"),
                    qktp.tile([P, N], F16, name=f"KT{pair}", tag="kt"),
                )
            QT, KT = qk_tiles[pair]
            for ch in chunks if chunks is not None else range(NCH):
                for w16, w8, dst, bias in (
                    (wq16_sb, wq8_sb, QT, bq_sb),
                    (wk16_sb, wk8_sb, KT, bk_sb),
                ):
                    if use_big_psum:
                        grp = psS.tile([P, 2, 512], F32, name="pj", tag="sg")
                        acc = grp[:, 0, :]
                    else:
                        acc = psProj.tile([P, 512], F32, name="pj", tag="proj")
                    if ch == 0:
                        for co in range(NCO):
                            nc.tensor.matmul(
                                acc,
                                lhsT=w16[:, co, ts(pair, P)],
                                rhs=xT16[:, co, :],
                                start=(co == 0),
                                stop=(co == NCO - 1),
                            )
                            yield
                    else:
                        for cp in range(NDR):
                            nc.tensor.matmul(
                                acc,
                                lhsT=w8[:, cp, :, ts(pair, P)],
                                rhs=xT8[:, cp, :, ts(ch - 1, 512)],
                                start=(cp == 0),
                                stop=(cp == NDR - 1),
                                perf_mode=DRM,
                            )
                            yield
                    nc.vector.tensor_scalar_add(
                        out=dst[:, ts(ch, 512)],
                        in0=acc,
                        scalar1=bias[:, pair : pair + 1],
                    )
                    yield "end"

        def vproj_gen(jt0, jt1, use_big_psum):
            for jt in range(jt0, jt1):
                if use_big_psum:
                    grp = psS.tile([P, 2, 512], F32, name="vps", tag="sg")
                    acc = grp[:, 0, :]
                else:
                    acc = psProj.tile([P, 512], F32, name="vps", tag="proj")
                if jt < 4:
                    for co in range(NCO):
                        nc.tensor.matmul(
                            acc,
                            lhsT=xT16[:, co, ts(jt, P)],
                            rhs=wv16_sb[:, co, :],
                            start=(co == 0),
                            stop=(co == NCO - 1),
                        )
                        yield
                    nc.vector.tensor_copy(
                        out=Vp16[:, jt, :, 0:DH],
                        in_=acc.rearrange("p (h f) -> p h f", h=8),
                    )
                else:
                    for cp in range(NDR):
                        nc.tensor.matmul(
                            acc,
                            lhsT=xT8[:, cp, :, ts(jt - 4, P)],
                            rhs=wv8_sb[:, cp, :, :],
                            start=(cp == 0),
                            stop=(cp == NDR - 1),
                            perf_mode=DRM,
                        )
                        yield
                nc.vector.tensor_copy(
                    out=Vp8[:, jt // 2, jt % 2, :, 0:DH],
                    in_=acc.rearrange("p (h f) -> p h f", h=8),
                )
                yield "end"

        def chain(*gens):
            for g in gens:
                yield from g

        class Fill:
            """Dispenses filler ops; a PSUM-accumulator group must never
            straddle an attention chunk boundary (its DVE eviction would
            queue behind the next chunk's PV start while PV waits on the
            slot that eviction frees -> deadlock)."""

            def __init__(self, gen, count0=0):
                self.gen = gen
                self.in_group = False
                self.count = count0

            def _next(self):
                v = next(self.gen, StopIteration)
                if v is StopIteration:
                    self.gen = None
                    self.in_group = False
                    return False
                self.count += 1
                self.in_group = v != "end"
                return True

            def append_gen(self, g):
                self.gen = chain(self.gen, g) if self.gen is not None else g

            def pull(self, n):
                for _ in range(n):
                    if self.gen is None or not self._next():
                        return

            def pull_to(self, target):
                while self.count < target and self.gen is not None:
                    if not self._next():
                        return

            def finish_group(self):
                while self.gen is not None and self.in_group:
                    self._next()

            def drain(self):
                while self.gen is not None and self._next():
                    pass

        def attn_chunk0(pair, fill, fill_rate, pre_pv=None):
            """Chunk 0, all-fp16 (short causal rows need the precision)."""
            QT, KT = qk_tiles[pair]
            hA, hB = 2 * pair, 2 * pair + 1
            oA = psO.tile([P, 512], F32, name="oA", tag="oA")
            oB = psO.tile([P, 512], F32, name="oB", tag="oB")
            pend = []

            def pv1(pt, jt, oP, h, h01):
                plo = P * jt if jt > 0 else 0
                nc.tensor.matmul(
                    oP[0 : DH + 1, plo:512],
                    lhsT=Vp16[:, jt, h, :],
                    rhs=pt[:, h01, plo:512],
                    start=(jt == 0),
                    stop=(jt == 3),
                )

            def pv_pair(a, b):
                for oP, h, h01 in ((oA, hA, 0), (oB, hB, 1)):
                    pv1(a[0], a[1], oP, h, h01)
                    pv1(b[0], b[1], oP, h, h01)

            for jt2 in range(0, 4, 2):
                if fill is not None:
                    fill.pull(fill_rate)
                for jt in (jt2, jt2 + 1):
                    sg = psS.tile([P, 2, 512], F32, name="sg", tag="sg")
                    lo = P * jt if jt > 0 else 0
                    nc.tensor.matmul(
                        sg[:, 0, lo:512],
                        lhsT=KT[0:DH, ts(jt, P)],
                        rhs=QT[0:DH, lo:512],
                        start=True,
                        stop=True,
                    )
                    nc.tensor.matmul(
                        sg[:, 1, lo:512],
                        lhsT=KT[DH:P, ts(jt, P)],
                        rhs=QT[DH:P, lo:512],
                        start=True,
                        stop=True,
                    )
                    pt = pexp.tile([P, 2, 512], F16, name="pt", tag="pt")
                    nc.scalar.activation(
                        out=pt[:, :, lo:512], in_=sg[:, :, lo:512], func=EXP,
                        scale=SCALE, bias=ebias,
                    )
                    nc.gpsimd.affine_select(
                        out=pt[:, :, lo : lo + P],
                        in_=pt[:, :, lo : lo + P],
                        compare_op=GE,
                        fill=0.0,
                        base=0,
                        channel_multiplier=-1,
                        pattern=[[0, 2], [1, P]],
                    )
                    pend.append((pt, jt))
            if pre_pv is not None:
                pre_pv()
            while pend:
                a = pend.pop(0)
                b = pend.pop(0)
                pv_pair(a, b)
            if fill is not None:
                fill.finish_group()
            return oA, oB

        def attn_chunk_dr(pair, ch, fill, fill_rate):
            """Chunks 1-3: fp16 S -> e4m3 pt -> DoubleRow PV over j-tile
            pairs.  Diagonal pairs: (r0,r1) full width with r1's cols
            [0:128] affine-zeroed; (r2,r3) cols [256:512] with r3's
            [256:384] affine-zeroed."""
            QT, KT = qk_tiles[pair]
            hA, hB = 2 * pair, 2 * pair + 1
            oA = psO.tile([P, 512], F32, name="oA", tag="oA")
            oB = psO.tile([P, 512], F32, name="oB", tag="oB")
            njp = 2 * ch + 2
            pend = []

            def dr_pv1(pt, jp, h01):
                plo = 256 if jp == njp - 1 else 0
                oP, h = ((oA, hA), (oB, hB))[h01]
                nc.tensor.matmul(
                    oP[0:VW, plo:512],
                    lhsT=Vp8[:, jp, :, h, :],
                    rhs=pt[:, :, h01, plo:512],
                    start=(jp == 0),
                    stop=(jp == njp - 1),
                    perf_mode=DRM,
                )

            for jp in range(njp):
                if fill is not None:
                    fill.pull(fill_rate)
                # full pairs on the DVE cadence use the Schraudolph exp
                # (one tensor_scalar producing e5m2 bits) to offload ScalarE
                dve = DVE_MOD > 0 and jp < 2 * ch and jp % DVE_MOD == 1
                pt = pexp.tile([P, 2, 2, 512], F8E5 if dve else F8,
                               name="pt", tag="pt")
                for jh in (0, 1):
                    jt = 2 * jp + jh
                    r = jt - 4 * ch
                    lo = P * r if r > 0 else 0
                    sg = psS.tile([P, 2, 512], F32, name="sg", tag="sg")
                    nc.tensor.matmul(
                        sg[:, 0, lo:512],
                        lhsT=KT[0:DH, ts(jt, P)],
                        rhs=QT[0:DH, 512 * ch + lo : 512 * (ch + 1)],
                        start=True,
                        stop=True,
                    )
                    nc.tensor.matmul(
                        sg[:, 1, lo:512],
                        lhsT=KT[DH:P, ts(jt, P)],
                        rhs=QT[DH:P, 512 * ch + lo : 512 * (ch + 1)],
                        start=True,
                        stop=True,
                    )
                    if dve:
                        nc.vector.tensor_scalar(
                            out=pt[:, jh, :, :].bitcast(U8), in0=sg,
                            scalar1=SCH_A, scalar2=SCH_B,
                            op0=mybir.AluOpType.mult,
                            op1=mybir.AluOpType.add,
                        )
                    else:
                        nc.scalar.activation(
                            out=pt[:, jh, :, lo:512], in_=sg[:, :, lo:512],
                            func=EXP, scale=SCALE, bias=ebias,
                        )
                    if r >= 0:
                        # keep where q >= 128*r + p; odd-r tiles are the
                        # second half of a DR pair whose matmul starts 128
                        # cols earlier -- zero that uncomputed strip
                        nc.gpsimd.affine_select(
                            out=pt[:, jh, :, lo : lo + P],
                            in_=pt[:, jh, :, lo : lo + P],
                            compare_op=GE,
                            fill=0.0,
                            base=0,
                            channel_multiplier=-1,
                            pattern=[[0, 2], [1, P]],
                        )
                        if r % 2 == 1:
                            nc.gpsimd.memset(pt[:, jh, :, lo - P : lo], 0.0)

                pend.append((pt, jp))
                while len(pend) > 1:
                    a = pend.pop(0)
                    dr_pv1(*a, 0)
                    dr_pv1(*a, 1)
            while pend:
                a = pend.pop(0)
                dr_pv1(*a, 0)
                dr_pv1(*a, 1)
            if fill is not None:
                fill.finish_group()
            return oA, oB

        def attn_emit(pair, fill, after_chunk=None, ascending=False,
                      pre_chunk=None, fill_rate=4, pre_pv=None):
            chunk_order = (
                list(range(NCH)) if ascending else list(range(NCH - 1, -1, -1))
            )
            for chi, ch in enumerate(chunk_order):
                if pre_chunk is not None:
                    pre_chunk(ch)
                if after_chunk is not None and chi > 0:
                    after_chunk(chunk_order[chi - 1])
                if ch == 0:
                    oA, oB = attn_chunk0(pair, fill, fill_rate, pre_pv=pre_pv)
                else:
                    oA, oB = attn_chunk_dr(pair, ch, fill, fill_rate)

                # ---- stage O' out of PSUM, then normalize off-path --------
                st = stgp.tile([DH + 1, 2, 512], F16, name="st", tag="st")
                # split staging across ScalarE+DVE: the PSUM banks (and the
                # next chunk's PV) unblock in half the time
                nc.scalar.copy(out=st[:, 0, :], in_=oA[0 : DH + 1, :])
                nc.vector.tensor_copy(out=st[:, 1, :], in_=oB[0 : DH + 1, :])
                # Reciprocal of the 1024 sums at full DVE lane utilization:
                # DMA-repack the denominator row onto 128 partitions, recip
                # there, unpack to a partition-0 row, then gpsimd-broadcast
                # it down 64 partitions.
                pk = rrp.tile([P, 8], F16, name="pk", tag="pk")
                rrow = rrp.tile([1, 2, 512], F16, name="rrow", tag="rrow")
                Rs = rrp.tile([DH, 2, 512], F16, name="Rs", tag="Rs")
                nc.sync.dma_start(out=pk, in_=st[DH : DH + 1, :, :])
                nc.vector.reciprocal(out=pk, in_=pk)
                nc.gpsimd.dma_start(out=rrow, in_=pk)
                nc.gpsimd.partition_broadcast(Rs, rrow[0:1, :, :])
                nc.vector.tensor_mul(
                    out=OT[0:DH, pair, ts(ch, 512)], in0=st[0:DH, 0, :],
                    in1=Rs[:, 0, :],
                )
                nc.vector.tensor_mul(
                    out=OT[DH:P, pair, ts(ch, 512)], in0=st[0:DH, 1, :],
                    in1=Rs[:, 1, :],
                )
                if after_chunk is not None and chi == NCH - 1:
                    after_chunk(ch)
            if fill is not None:
                fill.drain()

        def outproj_gen(ch, tail=False):
            for it in range(4 * ch, 4 * ch + 4):
                ob = outsb.tile([P, 2, 512], F16, name="ob", tag="ob")
                for e in range(2):
                    acc = psProj.tile([P, 512], F32, name="ops", tag="proj")
                    for p4 in range(NPAIR):
                        nc.tensor.matmul(
                            acc,
                            lhsT=OT[:, p4, ts(it, P)],
                            rhs=wo_sb[:, p4, ts(e, 512)],
                            start=(p4 == 0),
                            stop=(p4 == NPAIR - 1),
                        )
                        yield
                    # the tail chunk evicts on ScalarE (exp is done by then);
                    # mid-attention chunks use DVE (gpsimd can't read PSUM)
                    if tail and e == 0:
                        nc.scalar.copy(out=ob[:, e, :], in_=acc)
                    else:
                        nc.vector.tensor_copy(out=ob[:, e, :], in_=acc)
                    if e == 1:
                        nc.sync.dma_start(out=out_d[ts(it, P), :], in_=ob)
                    yield "end"

        # ---- emission schedule ------------------------------------------
        # upfront: only chunk 0 of pair-0's Q/K (fp16) and the first 4 V
        # j-tiles (fp16); pair-0 attention then runs ascending-chunk with a
        # fill chain of per-chunk blocks [Q ch, K ch, V 4ch..4ch+4] --
        # chunk ch only needs Q/K chunks <= ch and V j-tiles < 4ch+4,
        # guaranteed emitted by pull_to before the chunk starts -- followed
        # by pair-1's projections.
        for _ in qkproj_gen(0, use_big_psum=True, chunks=[0]):
            pass
        for _ in vproj_gen(0, 4, use_big_psum=True):
            pass

        OT = shared.tile([P, NPAIR, N], F16, name="OT", tag="ot")

        def primed(gen):
            next(gen)
            return gen

        def pair0_fill_gen():
            for ch in range(1, NCH):
                yield from qkproj_gen(0, False, chunks=[ch])
                yield from vproj_gen(4 * ch, 4 * ch + 4, False)
            yield from qkproj_gen(1, False)

        # fill-block op counts: vproj(0,4) = 36, then per-chunk blocks of
        # 30 (DR qk = 10 + DR vproj = 20)
        fill0 = Fill(primed(pair0_fill_gen()), count0=1)
        attn_emit(
            0,
            fill0,
            ascending=True,
            pre_chunk=lambda ch: fill0.pull_to(30 * ch),
            fill_rate=8,
        )
        attn_emit(1, Fill(primed(qkproj_gen(2, use_big_psum=False)), count0=1))
        attn_emit(2, Fill(primed(qkproj_gen(3, use_big_psum=False)), count0=1))

        # wo reuses wq16's slot (wq16 dead after pair-3 projections)
        wo_sb = shared.tile([P, NPAIR, D], F16, name="wo_sb", tag="wq_wo")
        for o4 in range(NPAIR):
            nc.sync.dma_start(out=wo_sb[:, o4, :], in_=wor[:, o4, :])

        # pair 3 with the output projection streaming through its fill
        # slots: each finished (normalized) chunk's outproj ops are appended
        # to the fill and drawn between S/PV groups
        fill3 = Fill(iter(()))
        attn_emit(
            3, fill3,
            after_chunk=lambda ch: fill3.append_gen(
                outproj_gen(ch, tail=(ch == 0))
            ),
            fill_rate=8,
        )


def build():
    nc = bacc.Bacc("TRN2", target_bir_lowering=False, debug=False, num_devices=8)
    xT0_d = nc.dram_tensor("xT0", [D, 512], F16, kind="ExternalInput").ap()
    x8_d = nc.dram_tensor("x8", [P, NDR, 2, N - 512], U8, kind="ExternalInput").ap()
    wq16_d = nc.dram_tensor("wq16", [D, DLOC], F16, kind="ExternalInput").ap()
    wq8_d = nc.dram_tensor("wq8", [P, NDR, 2, DLOC], U8, kind="ExternalInput").ap()
    wk16_d = nc.dram_tensor("wk16", [D, DLOC], F16, kind="ExternalInput").ap()
    wk8_d = nc.dram_tensor("wk8", [P, NDR, 2, DLOC], U8, kind="ExternalInput").ap()
    wv16_d = nc.dram_tensor("wv16", [D, DLOC], F16, kind="ExternalInput").ap()
    wv8_d = nc.dram_tensor("wv8", [P, NDR, 2, DLOC], U8, kind="ExternalInput").ap()
    wo_d = nc.dram_tensor("wo", [DLOC, D], F16, kind="ExternalInput").ap()
    bq_d = nc.dram_tensor("bq", [P, NPAIR], F32, kind="ExternalInput").ap()
    bk_d = nc.dram_tensor("bk", [P, NPAIR], F32, kind="ExternalInput").ap()
    out_d = nc.dram_tensor("out", [N, D], F16, kind="ExternalOutput").ap()
    with tile.TileContext(nc) as tc:
        _emit_kernel(tc, xT0_d, x8_d, wq16_d, wq8_d, wk16_d, wk8_d, wv16_d,
                     wv8_d, wo_d, bq_d, bk_d, out_d)
    nc.compile()
    return nc


_NC = None


def _get_nc():
    global _NC
    if _NC is None:
        _NC = build()
    return _NC


def _to_dr8(w):
    """[1024, F] fp32 -> [128, 4, 2, F] e4m3 bits (d = (2*cp + i)*128 + p)."""
    f = w.shape[1]
    return np.ascontiguousarray(
        w.reshape(NDR, 2, P, f).transpose(2, 0, 1, 3)
        .astype(ml_dtypes.float8_e4m3).view(np.uint8)
    )


def make_in_maps(x, Wq, bq, Wkv, bkv, Wo, bo):
    x = np.asarray(x, dtype=np.float32)
    Wq = np.asarray(Wq, dtype=np.float32)
    bq = np.asarray(bq, dtype=np.float32)
    Wkv = np.asarray(Wkv, dtype=np.float32)
    bkv = np.asarray(bkv, dtype=np.float32)
    Wo = np.asarray(Wo, dtype=np.float32)

    in_maps = []
    for c in range(8):
        bi, hj = c // 2, c % 2
        sl = slice(hj * DLOC, (hj + 1) * DLOC)
        slv = slice(D + hj * DLOC, D + (hj + 1) * DLOC)
        xT = np.ascontiguousarray(x[bi].T)
        in_maps.append(
            {
                "xT0": xT[:, 0:512].astype(np.float16),
                "x8": _to_dr8(xT[:, 512:]),
                "wq16": np.ascontiguousarray(Wq[:, sl]).astype(np.float16),
                "wq8": _to_dr8(Wq[:, sl]),
                "wk16": np.ascontiguousarray(Wkv[:, sl]).astype(np.float16),
                "wk8": _to_dr8(Wkv[:, sl]),
                "wv16": np.ascontiguousarray(Wkv[:, slv]).astype(np.float16),
                "wv8": _to_dr8(Wkv[:, slv]),
                "wo": np.ascontiguousarray(Wo[sl, :]).astype(np.float16),
                "bq": np.ascontiguousarray(bq[sl].reshape(NPAIR, P).T),
                "bk": np.ascontiguousarray(bkv[sl].reshape(NPAIR, P).T),
            }
        )
    return in_maps


def combine_outputs(results, bkv, Wo, bo):
    bo = np.asarray(bo, dtype=np.float32)
    bv = np.asarray(bkv, dtype=np.float32)[D:]
    c = bv @ np.asarray(Wo, dtype=np.float32) + bo
    outs = [results[cid]["out"].astype(np.float32) for cid in range(8)]
    full = np.stack([outs[2 * bi] + outs[2 * bi + 1] for bi in range(4)])
    return (full + c[None, None, :]).astype(np.float32)


def kernel(x, Wq, bq, Wkv, bkv, Wo, bo, **_ignored):
    nc = _get_nc()
    in_maps = make_in_maps(x, Wq, bq, Wkv, bkv, Wo, bo)
    res = bass_utils.run_bass_kernel_spmd(nc, in_maps, core_ids=list(range(8)))
    return combine_outputs(res.results, bkv, Wo, bo)
